# revision 5
# baseline (speedup 1.0000x reference)
"""Trainium2 Bass kernel for nn_AttentionBlock (GroupNorm -> QKV -> cross+self
attention -> back projection + residual).

Sharding: data-parallel over batch B=8, one batch element per NeuronCore.

Per-core math (C=512, T=1024, S=1024, 8 heads of 64):
  x   [512,1024] f32  -> GroupNorm(32 groups) -> xn bf16
  q/k = WqT/WkT @ xn        -> [512, 1024] bf16   (c_out = 64h+j on partitions)
  kc  = WkcT @ cond         -> [512, 1024] bf16
  vT  = xn.T @ WvT, cond.T @ WvcT -> 16 x [128, 8, 65] bf16 (65th col = ones)
  scores^T[s,t] = k[c,s]^T q[c,t] per head (transposed layout: no P transpose
  needed for PV). exp on ACT with the softmax scale (1/8) folded in. PV with
  the augmented ones-column produces Z (softmax denominator) as psum row 64.
  attn = PV * (1/Z broadcast) -> back proj (WbT) + bb + residual.
"""

import contextlib
import functools

import numpy as np
import ml_dtypes

import concourse.bacc as bacc
import concourse.bass as bass
import concourse.tile as tile
from concourse import mybir
from concourse import bass_utils

BF16 = ml_dtypes.bfloat16
F32 = mybir.dt.float32
BF = mybir.dt.bfloat16
AF = mybir.ActivationFunctionType
ALU = mybir.AluOpType
AX = mybir.AxisListType

C = 512
T = 1024
S = 1024
NH = 8
HS = 64
EPS = 1e-5
NK = 4          # 128-partition channel chunks
NSC = 16        # 128-row score s-chunks (self 0..7, cond 8..15)
GSIZE = 16      # channels per group


def _build_body(nc, tc, d, sbuf):
    pers = sbuf.enter_context(tc.tile_pool(name="pers", bufs=1))
    work = sbuf.enter_context(tc.tile_pool(name="work", bufs=2))
    epool = sbuf.enter_context(tc.tile_pool(name="epool", bufs=4))
    rzpool = sbuf.enter_context(tc.tile_pool(name="rzpool", bufs=2))
    outp = sbuf.enter_context(tc.tile_pool(name="outp", bufs=2))

    # ---------------- loads ----------------
    x_sb = []
    for j in range(NK):
        t_ = pers.tile([128, T], F32, tag=f"x{j}", name=f"x_sb{j}")
        nc.sync.dma_start(t_[:], d["x"][128 * j:128 * (j + 1), :])
        x_sb.append(t_)
    cond_sb = []
    for j in range(NK):
        t_ = pers.tile([128, S], BF, tag=f"cond{j}", name=f"cond_sb{j}")
        nc.sync.dma_start(t_[:], d["cond"][128 * j:128 * (j + 1), :])
        cond_sb.append(t_)

    def load_w(key):
        tiles = []
        for kk in range(NK):
            t_ = pers.tile([128, 512], BF, tag=f"{key}{kk}", name=f"{key}_sb{kk}")
            nc.sync.dma_start(t_[:], d[key][128 * kk:128 * (kk + 1), :])
            tiles.append(t_)
        return tiles

    wq_sb = load_w("wq")
    wk_sb = load_w("wk")
    wkc_sb = load_w("wkc")
    wv_sb = load_w("wv")
    wvc_sb = load_w("wvc")
    wb_sb = load_w("wb")

    def load_small(key, shape):
        t_ = pers.tile(shape, F32, tag=key, name=f"{key}_sb")
        nc.sync.dma_start(t_[:], d[key][:])
        return t_

    gamma_sb = load_small("gamma", [128, 4])
    beta_sb = load_small("beta", [128, 4])
    bq_sb = load_small("bq", [128, 4])
    bk_sb = load_small("bk", [128, 4])
    bkc_sb = load_small("bkc", [128, 4])
    bb_sb = load_small("bb", [128, 4])
    sel_f = load_small("sel_f", [128, 8])
    sel_b = load_small("sel_b", [8, 128])

    # v-biases broadcast across partitions (adds the bias to v before PV)
    bvb = pers.tile([128, 512], F32, tag="bvb", name="bvb")
    src = d["bv"][:]
    nc.sync.dma_start(bvb[:], bass.AP(tensor=src.tensor, offset=src.offset,
                                      ap=[[0, 128], [1, 512]]))
    bvcb = pers.tile([128, 512], F32, tag="bvcb", name="bvcb")
    src = d["bvc"][:]
    nc.sync.dma_start(bvcb[:], bass.AP(tensor=src.tensor, offset=src.offset,
                                       ap=[[0, 128], [1, 512]]))

    epsc = pers.tile([128, 1], F32, tag="epsc", name="epsc")
    nc.vector.memset(epsc[:], EPS)

    # ---------------- GroupNorm + projections (psum phase 1) ----------------
    with tc.tile_pool(name="ps1", bufs=4, space="PSUM") as ps1:
        stats = pers.tile([128, 8], F32, tag="stats", name="stats")
        for j in range(NK):
            scratch = work.tile([128, T], F32, tag="sq", name=f"sq{j}")
            nc.scalar.activation(scratch[:], x_sb[j][:], AF.Square,
                                 accum_out=stats[:, 4 + j:5 + j])
            nc.vector.reduce_sum(stats[:, j:j + 1], x_sb[j][:], axis=AX.X)

        gps = ps1.tile([8, 8], F32, tag="gn", bufs=2, name="gps")
        nc.tensor.matmul(gps[:], sel_f[:], stats[:], start=True, stop=True)
        gstats = pers.tile([8, 8], F32, tag="gstats", name="gstats")
        inv_n = 1.0 / (GSIZE * T)
        nc.vector.tensor_scalar_mul(gstats[:, 0:4], gps[:, 0:4], inv_n)  # mean
        nc.vector.tensor_scalar_mul(gstats[:, 4:8], gps[:, 4:8], inv_n)  # E[x^2]
        var = pers.tile([8, 4], F32, tag="var", name="var")
        nc.vector.tensor_mul(var[:], gstats[:, 0:4], gstats[:, 0:4])
        nc.vector.tensor_sub(var[:], gstats[:, 4:8], var[:])
        nc.scalar.activation(var[:], var[:], AF.Sqrt, bias=epsc[0:8, :])  # std
        nc.vector.reciprocal(gstats[:, 4:8], var[:])                      # rstd
        bps = ps1.tile([128, 8], F32, tag="gn", bufs=2, name="bps")
        nc.tensor.matmul(bps[:], sel_b[:], gstats[:], start=True, stop=True)
        scale = pers.tile([128, 4], F32, tag="scale", name="scale")
        shift = pers.tile([128, 4], F32, tag="shift", name="shift")
        nc.vector.tensor_mul(scale[:], gamma_sb[:], bps[:, 4:8])
        nc.vector.tensor_mul(shift[:], bps[:, 0:4], scale[:])
        nc.vector.tensor_sub(shift[:], beta_sb[:], shift[:])

        xn_sb = []
        for j in range(NK):
            t_ = pers.tile([128, T], BF, tag=f"xn{j}", name=f"xn_sb{j}")
            nc.vector.tensor_scalar(t_[:], x_sb[j][:], scale[:, j:j + 1],
                                    shift[:, j:j + 1], op0=ALU.mult, op1=ALU.add)
            xn_sb.append(t_)

        # -------- projections: q, k, kc (out: [c_out=64h+j, t] bf16) --------
        def proj(w_tiles, rhs_tiles, bias_sb, nm):
            outs = []
            for m in range(4):
                o = pers.tile([128, T], BF, tag=f"{nm}{m}", name=f"{nm}_sb{m}")
                outs.append(o)
            for m in range(4):
                for t2 in range(2):
                    ps = ps1.tile([128, 512], F32, tag="proj",
                                  name=f"ps_{nm}{m}{t2}")
                    for kk in range(NK):
                        nc.tensor.matmul(
                            ps[:], w_tiles[kk][:, 128 * m:128 * (m + 1)],
                            rhs_tiles[kk][:, 512 * t2:512 * (t2 + 1)],
                            start=(kk == 0), stop=(kk == NK - 1))
                    nc.vector.tensor_scalar(
                        outs[m][:, 512 * t2:512 * (t2 + 1)], ps[:],
                        bias_sb[:, m:m + 1], None, op0=ALU.add)
            return outs

        q_sb = proj(wq_sb, xn_sb, bq_sb, "q")
        k_sb = proj(wk_sb, xn_sb, bk_sb, "k")
        kc_sb = proj(wkc_sb, cond_sb, bkc_sb, "kc")

        # -------- vT: [s-chunk][128, head, 65] with ones col for Z --------
        vt_sb = []
        for i in range(NSC):
            t_ = pers.tile([128, 8, 65], BF, tag=f"vt{i}", name=f"vt_sb{i}")
            nc.vector.memset(t_[:, :, 64:65], 1.0)
            vt_sb.append(t_)
        for i in range(NSC):
            if i < 8:
                src, w, bcast = xn_sb, wv_sb, bvb
            else:
                src, w, bcast = cond_sb, wvc_sb, bvcb
            m8 = i % 8
            ps = ps1.tile([128, 512], F32, tag="proj", name=f"ps_vt{i}")
            for kk in range(NK):
                nc.tensor.matmul(ps[:], src[kk][:, 128 * m8:128 * (m8 + 1)],
                                 w[kk][:], start=(kk == 0), stop=(kk == NK - 1))
            for h in range(NH):
                nc.vector.tensor_add(vt_sb[i][:, h, 0:64],
                                     ps[:, 64 * h:64 * (h + 1)],
                                     bcast[:, 64 * h:64 * (h + 1)])

    # ---------------- attention (psum phase 2) ----------------
    attn_sb = []
    for p in range(4):
        t_ = pers.tile([128, T], BF, tag=f"attn{p}", name=f"attn_sb{p}")
        attn_sb.append(t_)

    with tc.tile_pool(name="ps_sc", bufs=2, space="PSUM") as ps_sc, \
         tc.tile_pool(name="ps_pv", bufs=1, space="PSUM") as ps_pv, \
         tc.tile_pool(name="zdram", bufs=2, space="DRAM") as zdram:
        for p in range(4):
            pvs = []
            for j in range(4):  # j = h_idx*2 + t2
                t_ = ps_pv.tile([65, 512], F32, tag=f"pv{j}", name=f"pv{p}_{j}")
                pvs.append(t_)
            for i in range(NSC):
                ksrc = k_sb[p] if i < 8 else kc_sb[p]
                scol = 128 * (i % 8)
                e_tiles = []
                for h_idx, rb in ((0, 0), (1, 64)):
                    sc = ps_sc.tile([128, T], F32, tag="sc",
                                    name=f"sc{p}_{i}_{h_idx}")
                    for t2 in range(2):
                        nc.tensor.matmul(
                            sc[:, 512 * t2:512 * (t2 + 1)],
                            ksrc[rb:rb + 64, scol:scol + 128],
                            q_sb[p][rb:rb + 64, 512 * t2:512 * (t2 + 1)],
                            start=True, stop=True)
                    e = epool.tile([128, T], BF, tag="e", name=f"e{p}_{i}_{h_idx}")
                    nc.scalar.activation(e[:], sc[:], AF.Exp, scale=0.125)
                    e_tiles.append(e)
                for h_idx in range(2):
                    h = 2 * p + h_idx
                    for t2 in range(2):
                        nc.tensor.matmul(pvs[2 * h_idx + t2][:],
                                         vt_sb[i][:, h, :],
                                         e_tiles[h_idx][:, 512 * t2:512 * (t2 + 1)],
                                         start=(i == 0), stop=(i == NSC - 1))
            # softmax denominators: Z sits in row 64 of each pv psum tile.
            # DMA can't read PSUM -> bounce via SBUF, then broadcast-DMA.
            zsb = rzpool.tile([128, 2048], F32, tag="zsb", name=f"zsb{p}")
            for j in range(4):
                nc.vector.tensor_copy(zsb[64:65, 512 * j:512 * (j + 1)],
                                      pvs[j][64:65, :])
            zd = zdram.tile([1, 2048], F32, tag="zd", name=f"zd{p}")
            nc.sync.dma_start(zd[:], zsb[64:65, :])
            rzb = rzpool.tile([128, T], F32, tag="rzb", name=f"rzb{p}")
            for j in range(4):
                h_idx, t2 = j // 2, j % 2
                zrow = zd[0:1, 512 * j:512 * (j + 1)]
                nc.sync.dma_start(
                    rzb[64 * h_idx:64 * (h_idx + 1), 512 * t2:512 * (t2 + 1)],
                    bass.AP(tensor=zrow.tensor, offset=zrow.offset,
                            ap=[[0, 64], [1, 512]]))
            nc.vector.reciprocal(rzb[:], rzb[:])
            for j in range(4):
                h_idx, t2 = j // 2, j % 2
                nc.vector.tensor_mul(
                    attn_sb[p][64 * h_idx:64 * (h_idx + 1),
                               512 * t2:512 * (t2 + 1)],
                    pvs[j][0:64, :],
                    rzb[64 * h_idx:64 * (h_idx + 1), 512 * t2:512 * (t2 + 1)])

    # ---------------- back projection + residual (psum phase 3) -------------
    with tc.tile_pool(name="ps_bk", bufs=2, space="PSUM") as ps_bk:
        for m in range(4):
            outsb = outp.tile([128, T], F32, tag="outsb", name=f"outsb{m}")
            for t2 in range(2):
                ps = ps_bk.tile([128, 512], F32, tag="bk", name=f"ps_bk{m}{t2}")
                for kk in range(NK):
                    nc.tensor.matmul(ps[:],
                                     wb_sb[kk][:, 128 * m:128 * (m + 1)],
                                     attn_sb[kk][:, 512 * t2:512 * (t2 + 1)],
                                     start=(kk == 0), stop=(kk == NK - 1))
                nc.vector.scalar_tensor_tensor(
                    outsb[:, 512 * t2:512 * (t2 + 1)], ps[:], bb_sb[:, m:m + 1],
                    x_sb[m][:, 512 * t2:512 * (t2 + 1)],
                    op0=ALU.add, op1=ALU.add)
            nc.sync.dma_start(d["out"][128 * m:128 * (m + 1), :], outsb[:])


@functools.lru_cache(maxsize=1)
def _build():
    nc = bacc.Bacc("TRN2", target_bir_lowering=False, debug=False)
    d = {}
    d["x"] = nc.dram_tensor("x", [C, T], F32, kind="ExternalInput")
    d["cond"] = nc.dram_tensor("cond", [512, S], BF, kind="ExternalInput")
    for w in ("wq", "wk", "wkc", "wv", "wvc", "wb"):
        d[w] = nc.dram_tensor(w, [512, 512], BF, kind="ExternalInput")
    for v in ("gamma", "beta", "bq", "bk", "bkc", "bb"):
        d[v] = nc.dram_tensor(v, [128, 4], F32, kind="ExternalInput")
    d["bv"] = nc.dram_tensor("bv", [1, 512], F32, kind="ExternalInput")
    d["bvc"] = nc.dram_tensor("bvc", [1, 512], F32, kind="ExternalInput")
    d["sel_f"] = nc.dram_tensor("sel_f", [128, 8], F32, kind="ExternalInput")
    d["sel_b"] = nc.dram_tensor("sel_b", [8, 128], F32, kind="ExternalInput")
    d["out"] = nc.dram_tensor("out", [C, T], F32, kind="ExternalOutput")

    with tile.TileContext(nc) as tc:
        with contextlib.ExitStack() as sbuf:
            _build_body(nc, tc, d, sbuf)
    nc.compile()
    return nc


def _prep_shared(gn_gamma, gn_beta, Wf, bf, Wt, bt, Wb, bb):
    f32 = np.float32
    Wf_r = np.asarray(Wf, f32).reshape(8, 3, 64, 512)
    Wt_r = np.asarray(Wt, f32).reshape(8, 2, 64, 512)
    bf_r = np.asarray(bf, f32).reshape(8, 3, 64)
    bt_r = np.asarray(bt, f32).reshape(8, 2, 64)

    def wT(a):  # [512(out), 512(in)] -> [in, out] bf16
        return np.ascontiguousarray(a.reshape(512, 512).T).astype(BF16)

    def pcol(v):  # [512] -> [128, 4]
        return np.ascontiguousarray(np.asarray(v, f32).reshape(4, 128).T)

    sel_f = (np.arange(128)[:, None] // GSIZE ==
             np.arange(8)[None, :]).astype(f32)
    return {
        "wq": wT(Wf_r[:, 0]),
        "wk": wT(Wf_r[:, 1]),
        "wv": wT(Wf_r[:, 2]),
        "wkc": wT(Wt_r[:, 0]),
        "wvc": wT(Wt_r[:, 1]),
        "wb": np.ascontiguousarray(np.asarray(Wb, f32).T).astype(BF16),
        "gamma": pcol(gn_gamma),
        "beta": pcol(gn_beta),
        "bq": pcol(bf_r[:, 0].reshape(512)),
        "bk": pcol(bf_r[:, 1].reshape(512)),
        "bkc": pcol(bt_r[:, 0].reshape(512)),
        "bb": pcol(bb),
        "bv": np.ascontiguousarray(bf_r[:, 2].reshape(1, 512)),
        "bvc": np.ascontiguousarray(bt_r[:, 1].reshape(1, 512)),
        "sel_f": sel_f,
        "sel_b": np.ascontiguousarray(sel_f.T),
    }


def _run(inputs, trace=False, tmpdir=None):
    nc = _build()
    shared = _prep_shared(inputs["gn_gamma"], inputs["gn_beta"],
                          inputs["Wf"], inputs["bf"], inputs["Wt"],
                          inputs["bt"], inputs["Wb"], inputs["bb"])
    feat = np.asarray(inputs["input_feature"], np.float32)
    cond = np.asarray(inputs["attention_condition"], np.float32)
    in_maps = []
    for b in range(8):
        m = dict(shared)
        m["x"] = np.ascontiguousarray(feat[b].reshape(C, T))
        m["cond"] = cond[b].astype(BF16)
        in_maps.append(m)
    res = bass_utils.run_bass_kernel_spmd(nc, in_maps, core_ids=list(range(8)),
                                          trace=trace, tmpdir=tmpdir)
    out = np.stack([r["out"] for r in res.results], axis=0)
    return out.reshape(8, C, 32, 32).astype(np.float32), res


def kernel(**inputs):
    out, _ = _run(inputs, trace=False)
    return out


# revision 8
# speedup vs baseline: 1.0532x; 1.0532x over previous
"""Trainium2 Bass kernel for nn_AttentionBlock (GroupNorm -> QKV -> cross+self
attention -> back projection + residual).

Sharding: data-parallel over batch B=8, one batch element per NeuronCore.

Per-core math (C=512, T=1024, S=1024, 8 heads of 64):
  x   [512,1024] f32  -> GroupNorm(32 groups) -> xn bf16
  q/k = WqT/WkT @ xn        -> [512, 1024] bf16   (c_out = 64h+j on partitions)
  kc  = WkcT @ cond         -> [512, 1024] bf16
  vT  = xn.T @ WvT, cond.T @ WvcT -> 16 x [128, 8, 65] bf16 (65th col = ones)
  scores^T[s,t] = k[c,s]^T q[c,t] per head (transposed layout: no P transpose
  needed for PV). exp on ACT with the softmax scale (1/8) folded in. PV with
  the augmented ones-column produces Z (softmax denominator) as psum row 64.
  attn = PV * (1/Z broadcast) -> back proj (WbT) + bb + residual.
"""

import contextlib
import functools

import numpy as np
import ml_dtypes

import concourse.bacc as bacc
import concourse.bass as bass
import concourse.tile as tile
from concourse import mybir
from concourse import bass_utils

BF16 = ml_dtypes.bfloat16
F32 = mybir.dt.float32
BF = mybir.dt.bfloat16
AF = mybir.ActivationFunctionType
ALU = mybir.AluOpType
AX = mybir.AxisListType

C = 512
T = 1024
S = 1024
NH = 8
HS = 64
EPS = 1e-5
NK = 4          # 128-partition channel chunks
NSC = 16        # 128-row score s-chunks (self 0..7, cond 8..15)
GSIZE = 16      # channels per group


def _build_body(nc, tc, d, sbuf):
    pers = sbuf.enter_context(tc.tile_pool(name="pers", bufs=1))
    work = sbuf.enter_context(tc.tile_pool(name="work", bufs=2))
    epool = sbuf.enter_context(tc.tile_pool(name="epool", bufs=4))
    rzpool = sbuf.enter_context(tc.tile_pool(name="rzpool", bufs=2))
    outp = sbuf.enter_context(tc.tile_pool(name="outp", bufs=2))

    # ---------------- loads ----------------
    # x + small tensors on the sync queue; weights on the gpsimd queue so
    # GroupNorm can start while weights stream in.
    x_sb = []
    for j in range(NK):
        t_ = pers.tile([128, T], F32, tag=f"x{j}", name=f"x_sb{j}")
        nc.sync.dma_start(t_[:], d["x"][128 * j:128 * (j + 1), :])
        x_sb.append(t_)
    cond_sb = []
    for j in range(NK):
        t_ = pers.tile([128, S], BF, tag=f"cond{j}", name=f"cond_sb{j}")
        nc.sync.dma_start(t_[:], d["cond"][128 * j:128 * (j + 1), :])
        cond_sb.append(t_)

    def load_w(key):
        tiles = []
        for kk in range(NK):
            t_ = pers.tile([128, 512], BF, tag=f"{key}{kk}", name=f"{key}_sb{kk}")
            nc.gpsimd.dma_start(t_[:], d[key][128 * kk:128 * (kk + 1), :])
            tiles.append(t_)
        return tiles

    wq_sb = load_w("wq")
    wk_sb = load_w("wk")
    wkc_sb = load_w("wkc")
    wv_sb = load_w("wv")
    wvc_sb = load_w("wvc")
    wb_sb = load_w("wb")

    def load_small(key, shape):
        t_ = pers.tile(shape, F32, tag=key, name=f"{key}_sb")
        nc.sync.dma_start(t_[:], d[key][:])
        return t_

    gamma_sb = load_small("gamma", [128, 4])
    beta_sb = load_small("beta", [128, 4])
    bq_sb = load_small("bq", [128, 4])
    bk_sb = load_small("bk", [128, 4])
    bkc_sb = load_small("bkc", [128, 4])
    bb_sb = load_small("bb", [128, 4])
    sel_f = load_small("sel_f", [128, 8])
    sel_b = load_small("sel_b", [8, 128])

    # v-biases broadcast across partitions (adds the bias to v before PV)
    bvb = pers.tile([128, 512], F32, tag="bvb", name="bvb")
    src = d["bv"][:]
    nc.sync.dma_start(bvb[:], bass.AP(tensor=src.tensor, offset=src.offset,
                                      ap=[[0, 128], [1, 512]]))
    bvcb = pers.tile([128, 512], F32, tag="bvcb", name="bvcb")
    src = d["bvc"][:]
    nc.sync.dma_start(bvcb[:], bass.AP(tensor=src.tensor, offset=src.offset,
                                       ap=[[0, 128], [1, 512]]))

    epsc = pers.tile([128, 1], F32, tag="epsc", name="epsc")
    nc.vector.memset(epsc[:], EPS)

    # ---------------- GroupNorm + projections (psum phase 1) ----------------
    with tc.tile_pool(name="ps1", bufs=4, space="PSUM") as ps1:
        stats = pers.tile([128, 8], F32, tag="stats", name="stats")
        for j in range(NK):
            scratch = work.tile([128, T], F32, tag="sq", name=f"sq{j}")
            nc.scalar.activation(scratch[:], x_sb[j][:], AF.Square,
                                 accum_out=stats[:, 4 + j:5 + j])
            nc.vector.reduce_sum(stats[:, j:j + 1], x_sb[j][:], axis=AX.X)

        gps = ps1.tile([8, 8], F32, tag="gn", bufs=2, name="gps")
        nc.tensor.matmul(gps[:], sel_f[:], stats[:], start=True, stop=True)
        gstats = pers.tile([8, 8], F32, tag="gstats", name="gstats")
        inv_n = 1.0 / (GSIZE * T)
        nc.vector.tensor_scalar_mul(gstats[:, 0:4], gps[:, 0:4], inv_n)  # mean
        nc.vector.tensor_scalar_mul(gstats[:, 4:8], gps[:, 4:8], inv_n)  # E[x^2]
        var = pers.tile([8, 4], F32, tag="var", name="var")
        nc.vector.tensor_mul(var[:], gstats[:, 0:4], gstats[:, 0:4])
        nc.vector.tensor_sub(var[:], gstats[:, 4:8], var[:])
        nc.scalar.activation(var[:], var[:], AF.Sqrt, bias=epsc[0:8, :])  # std
        nc.vector.reciprocal(gstats[:, 4:8], var[:])                      # rstd
        bps = ps1.tile([128, 8], F32, tag="gn", bufs=2, name="bps")
        nc.tensor.matmul(bps[:], sel_b[:], gstats[:], start=True, stop=True)
        scale = pers.tile([128, 4], F32, tag="scale", name="scale")
        shift = pers.tile([128, 4], F32, tag="shift", name="shift")
        nc.vector.tensor_mul(scale[:], gamma_sb[:], bps[:, 4:8])
        nc.vector.tensor_mul(shift[:], bps[:, 0:4], scale[:])
        nc.vector.tensor_sub(shift[:], beta_sb[:], shift[:])

        xn_sb = []
        for j in range(NK):
            t_ = pers.tile([128, T], BF, tag=f"xn{j}", name=f"xn_sb{j}")
            nc.vector.tensor_scalar(t_[:], x_sb[j][:], scale[:, j:j + 1],
                                    shift[:, j:j + 1], op0=ALU.mult, op1=ALU.add)
            xn_sb.append(t_)

        # -------- projections: q, k, kc (out: [c_out=64h+j, t] bf16) --------
        def proj(w_tiles, rhs_tiles, bias_sb, nm):
            outs = []
            for m in range(4):
                o = pers.tile([128, T], BF, tag=f"{nm}{m}", name=f"{nm}_sb{m}")
                outs.append(o)
            for m in range(4):
                for t2 in range(2):
                    ps = ps1.tile([128, 512], F32, tag="proj",
                                  name=f"ps_{nm}{m}{t2}")
                    for kk in range(NK):
                        nc.tensor.matmul(
                            ps[:], w_tiles[kk][:, 128 * m:128 * (m + 1)],
                            rhs_tiles[kk][:, 512 * t2:512 * (t2 + 1)],
                            start=(kk == 0), stop=(kk == NK - 1))
                    nc.vector.tensor_scalar(
                        outs[m][:, 512 * t2:512 * (t2 + 1)], ps[:],
                        bias_sb[:, m:m + 1], None, op0=ALU.add)
            return outs

        q_sb = proj(wq_sb, xn_sb, bq_sb, "q")
        k_sb = proj(wk_sb, xn_sb, bk_sb, "k")
        kc_sb = proj(wkc_sb, cond_sb, bkc_sb, "kc")

        # -------- vT: [s-chunk][128, head, 65] with ones col for Z --------
        vt_sb = []
        for i in range(NSC):
            t_ = pers.tile([128, 8, 65], BF, tag=f"vt{i}", name=f"vt_sb{i}")
            nc.vector.memset(t_[:, :, 64:65], 1.0)
            vt_sb.append(t_)
        for i in range(NSC):
            if i < 8:
                src, w, bcast = xn_sb, wv_sb, bvb
            else:
                src, w, bcast = cond_sb, wvc_sb, bvcb
            m8 = i % 8
            ps = ps1.tile([128, 512], F32, tag="proj", name=f"ps_vt{i}")
            for kk in range(NK):
                nc.tensor.matmul(ps[:], src[kk][:, 128 * m8:128 * (m8 + 1)],
                                 w[kk][:], start=(kk == 0), stop=(kk == NK - 1))
            nc.vector.tensor_add(
                vt_sb[i][:, :, 0:64],
                ps[:].rearrange("p (h c) -> p h c", h=NH),
                bcast[:].rearrange("p (h c) -> p h c", h=NH))

    # ---------------- attention (psum phase 2) ----------------
    attn_sb = []
    for p in range(4):
        t_ = pers.tile([128, T], BF, tag=f"attn{p}", name=f"attn_sb{p}")
        attn_sb.append(t_)

    with tc.tile_pool(name="ps_sc", bufs=2, space="PSUM") as ps_sc, \
         tc.tile_pool(name="ps_pv", bufs=1, space="PSUM") as ps_pv, \
         tc.tile_pool(name="zdram", bufs=2, space="DRAM") as zdram:
        for p in range(4):
            pvs = []
            for j in range(4):  # j = h_idx*2 + t2
                t_ = ps_pv.tile([65, 512], F32, tag=f"pv{j}", name=f"pv{p}_{j}")
                pvs.append(t_)
            for i in range(NSC):
                ksrc = k_sb[p] if i < 8 else kc_sb[p]
                scol = 128 * (i % 8)
                e_tiles = []
                for h_idx, rb in ((0, 0), (1, 64)):
                    sc = ps_sc.tile([128, T], F32, tag="sc",
                                    name=f"sc{p}_{i}_{h_idx}")
                    for t2 in range(2):
                        nc.tensor.matmul(
                            sc[:, 512 * t2:512 * (t2 + 1)],
                            ksrc[rb:rb + 64, scol:scol + 128],
                            q_sb[p][rb:rb + 64, 512 * t2:512 * (t2 + 1)],
                            start=True, stop=True)
                    e = epool.tile([128, T], BF, tag="e", name=f"e{p}_{i}_{h_idx}")
                    nc.scalar.activation(e[:], sc[:], AF.Exp, scale=0.125)
                    e_tiles.append(e)
                for h_idx in range(2):
                    h = 2 * p + h_idx
                    for t2 in range(2):
                        nc.tensor.matmul(pvs[2 * h_idx + t2][:],
                                         vt_sb[i][:, h, :],
                                         e_tiles[h_idx][:, 512 * t2:512 * (t2 + 1)],
                                         start=(i == 0), stop=(i == NSC - 1))
            # Drain pv psum FAST so the next pair's PV matmuls don't stall:
            # copy Z rows + unnormalized PV out to SBUF, normalize later.
            zsb = rzpool.tile([128, 2048], F32, tag="zsb", name=f"zsb{p}")
            for j in range(4):
                h_idx, t2 = j // 2, j % 2
                nc.vector.tensor_copy(
                    zsb[64:65, 1024 * h_idx + 512 * t2:
                        1024 * h_idx + 512 * (t2 + 1)],
                    pvs[j][64:65, :])
                nc.vector.tensor_copy(
                    attn_sb[p][64 * h_idx:64 * (h_idx + 1),
                               512 * t2:512 * (t2 + 1)],
                    pvs[j][0:64, :])
            # 1/Z: bounce via DRAM reshaped to [128, 16] so the (slow per
            # element) DVE reciprocal runs wide, then broadcast to rzb.
            zd = zdram.tile([1, 2048], F32, tag="zd", name=f"zd{p}")
            nc.sync.dma_start(zd[:], zsb[64:65, :])
            zr = rzpool.tile([128, 16], F32, tag="zr", name=f"zr{p}")
            nc.sync.dma_start(zr[:], zd[:].rearrange("o (p j) -> (o p) j", p=128))
            nc.vector.reciprocal(zr[:], zr[:])
            zd2 = zdram.tile([1, 2048], F32, tag="zd2", name=f"zd2{p}")
            nc.sync.dma_start(zd2[:].rearrange("o (p j) -> (o p) j", p=128), zr[:])
            rzb = rzpool.tile([128, T], F32, tag="rzb", name=f"rzb{p}")
            for h_idx in range(2):
                zrow = zd2[0:1, 1024 * h_idx:1024 * (h_idx + 1)]
                nc.sync.dma_start(
                    rzb[64 * h_idx:64 * (h_idx + 1), :],
                    bass.AP(tensor=zrow.tensor, offset=zrow.offset,
                            ap=[[0, 64], [1, 1024]]))
            nc.vector.tensor_mul(attn_sb[p][:], attn_sb[p][:], rzb[:])

    # ---------------- back projection + residual (psum phase 3) -------------
    with tc.tile_pool(name="ps_bk", bufs=2, space="PSUM") as ps_bk:
        for m in range(4):
            outsb = outp.tile([128, T], F32, tag="outsb", name=f"outsb{m}")
            for t2 in range(2):
                ps = ps_bk.tile([128, 512], F32, tag="bk", name=f"ps_bk{m}{t2}")
                for kk in range(NK):
                    nc.tensor.matmul(ps[:],
                                     wb_sb[kk][:, 128 * m:128 * (m + 1)],
                                     attn_sb[kk][:, 512 * t2:512 * (t2 + 1)],
                                     start=(kk == 0), stop=(kk == NK - 1))
                nc.vector.scalar_tensor_tensor(
                    outsb[:, 512 * t2:512 * (t2 + 1)], ps[:], bb_sb[:, m:m + 1],
                    x_sb[m][:, 512 * t2:512 * (t2 + 1)],
                    op0=ALU.add, op1=ALU.add)
            nc.sync.dma_start(d["out"][128 * m:128 * (m + 1), :], outsb[:])


@functools.lru_cache(maxsize=1)
def _build():
    nc = bacc.Bacc("TRN2", target_bir_lowering=False, debug=False)
    d = {}
    d["x"] = nc.dram_tensor("x", [C, T], F32, kind="ExternalInput")
    d["cond"] = nc.dram_tensor("cond", [512, S], BF, kind="ExternalInput")
    for w in ("wq", "wk", "wkc", "wv", "wvc", "wb"):
        d[w] = nc.dram_tensor(w, [512, 512], BF, kind="ExternalInput")
    for v in ("gamma", "beta", "bq", "bk", "bkc", "bb"):
        d[v] = nc.dram_tensor(v, [128, 4], F32, kind="ExternalInput")
    d["bv"] = nc.dram_tensor("bv", [1, 512], F32, kind="ExternalInput")
    d["bvc"] = nc.dram_tensor("bvc", [1, 512], F32, kind="ExternalInput")
    d["sel_f"] = nc.dram_tensor("sel_f", [128, 8], F32, kind="ExternalInput")
    d["sel_b"] = nc.dram_tensor("sel_b", [8, 128], F32, kind="ExternalInput")
    d["out"] = nc.dram_tensor("out", [C, T], F32, kind="ExternalOutput")

    with tile.TileContext(nc) as tc:
        with contextlib.ExitStack() as sbuf:
            _build_body(nc, tc, d, sbuf)
    nc.compile()
    return nc


def _prep_shared(gn_gamma, gn_beta, Wf, bf, Wt, bt, Wb, bb):
    f32 = np.float32
    Wf_r = np.asarray(Wf, f32).reshape(8, 3, 64, 512)
    Wt_r = np.asarray(Wt, f32).reshape(8, 2, 64, 512)
    bf_r = np.asarray(bf, f32).reshape(8, 3, 64)
    bt_r = np.asarray(bt, f32).reshape(8, 2, 64)

    def wT(a):  # [512(out), 512(in)] -> [in, out] bf16
        return np.ascontiguousarray(a.reshape(512, 512).T).astype(BF16)

    def pcol(v):  # [512] -> [128, 4]
        return np.ascontiguousarray(np.asarray(v, f32).reshape(4, 128).T)

    sel_f = (np.arange(128)[:, None] // GSIZE ==
             np.arange(8)[None, :]).astype(f32)
    return {
        "wq": wT(Wf_r[:, 0]),
        "wk": wT(Wf_r[:, 1]),
        "wv": wT(Wf_r[:, 2]),
        "wkc": wT(Wt_r[:, 0]),
        "wvc": wT(Wt_r[:, 1]),
        "wb": np.ascontiguousarray(np.asarray(Wb, f32).T).astype(BF16),
        "gamma": pcol(gn_gamma),
        "beta": pcol(gn_beta),
        "bq": pcol(bf_r[:, 0].reshape(512)),
        "bk": pcol(bf_r[:, 1].reshape(512)),
        "bkc": pcol(bt_r[:, 0].reshape(512)),
        "bb": pcol(bb),
        "bv": np.ascontiguousarray(bf_r[:, 2].reshape(1, 512)),
        "bvc": np.ascontiguousarray(bt_r[:, 1].reshape(1, 512)),
        "sel_f": sel_f,
        "sel_b": np.ascontiguousarray(sel_f.T),
    }


def _run(inputs, trace=False, tmpdir=None):
    nc = _build()
    shared = _prep_shared(inputs["gn_gamma"], inputs["gn_beta"],
                          inputs["Wf"], inputs["bf"], inputs["Wt"],
                          inputs["bt"], inputs["Wb"], inputs["bb"])
    feat = np.asarray(inputs["input_feature"], np.float32)
    cond = np.asarray(inputs["attention_condition"], np.float32)
    in_maps = []
    for b in range(8):
        m = dict(shared)
        m["x"] = np.ascontiguousarray(feat[b].reshape(C, T))
        m["cond"] = cond[b].astype(BF16)
        in_maps.append(m)
    res = bass_utils.run_bass_kernel_spmd(nc, in_maps, core_ids=list(range(8)),
                                          trace=trace, tmpdir=tmpdir)
    out = np.stack([r["out"] for r in res.results], axis=0)
    return out.reshape(8, C, 32, 32).astype(np.float32), res


def kernel(**inputs):
    out, _ = _run(inputs, trace=False)
    return out
